# revision 11
# baseline (speedup 1.0000x reference)
"""CTC loss forward on 8 TRN2 NeuronCores, data-parallel over batch.

Problem: log_probs (512, 32, 8000) f32, targets (32, 40) i32,
target_lengths (32,) i32 -> per-sample loss (32,) f32
(input_lengths is ignored, matching the reference).

Strategy per core (4 samples):
 - Gather only the needed log-prob entries: glp[s, t, n] = lp[t, n, et[n, s]]
   (T*4*81 = 166K elements) via one indirect DMA; the 512MB tensor is
   never streamed.
 - Run the T-step DP in linear probability space with an augmented state
   on partitions: rows 0..80 = alpha over the 81 CTC states, rows
   81..119 = the 39 masked skip terms am[j] = alpha[2j+1]*mask[2j+3].
   One constant 120x120 matmul performs all shifts AND regenerates the
   duplicated skip rows; one FD=4 DVE multiply by the precomputed
   per-step probability page completes the step:
       X' = (W2 @ X) * P2[:, t]
 - Every K=8 steps renormalize by the per-sample sum of alpha over
   states s <= 2L (window excludes padding states that run away),
   clamp, and log-accumulate the scales.
 - loss = -(log(alpha[2L] + alpha[2L-1]) + sum(log scales) - T*SHIFT)/L
"""
import sys

for _p in ("/opt/trn_rl_repo",):
    if _p not in sys.path:
        sys.path.append(_p)

import numpy as np
import concourse.bass as bass
import concourse.bacc as bacc
import concourse.mybir as mybir
from concourse import tile
from concourse.bass_utils import run_bass_kernel_spmd

F32 = mybir.dt.float32
I32 = mybir.dt.int32
BF = mybir.dt.bfloat16
AF = mybir.ActivationFunctionType
OP = mybir.AluOpType

T_FULL = 512
NL = 4          # samples per core
NC_CORES = 8
C = 8000
S = 40
SE = 2 * S + 1  # 81
NJ = 39         # skip rows: odd states 1,3,..,77
NP = SE + NJ    # 120 partitions of augmented state
K_RENORM = 16
SHIFT = 9.0
E_SHIFT = float(np.float32(np.exp(np.float32(SHIFT))))
CLAMP = 1e26


def _ap(t, off, dims):
    a = t[:]
    return bass.AP(a.tensor, off, [list(d) for d in dims])


def build_nc(T=T_FULL):
    nc = bacc.Bacc("TRN2", target_bir_lowering=False, debug=True)
    lp_ext = nc.declare_dram_parameter("log_probs", [T, NL, C], F32, isOutput=False)
    tg_ext = nc.declare_dram_parameter("targets", [NL, S], I32, isOutput=False)
    tl_ext = nc.declare_dram_parameter("target_lengths", [NL], I32, isOutput=False)
    out_ext = nc.declare_dram_parameter("out", [1, NL], F32, isOutput=True)

    n_ren = len([t for t in range(1, T) if t % K_RENORM == 0 and t != T - 1]) + 1

    with tile.TileContext(nc) as tc:
        with (
            tc.tile_pool(name="cst", bufs=1) as cst,
            tc.tile_pool(name="big", bufs=1) as big,
            tc.tile_pool(name="x", bufs=3) as xpool,
            tc.tile_pool(name="tmp", bufs=2) as tmp,
            tc.tile_pool(name="ps", bufs=2, space=bass.MemorySpace.PSUM) as psp,
            tc.tile_pool(name="ps1", bufs=2, space=bass.MemorySpace.PSUM) as ps1,
        ):
            # ---------- constants built on device ----------
            dmat = cst.tile([128, 128], I32, tag="dmat")
            nc.gpsimd.iota(dmat[:], pattern=[[1, 128]], base=0, channel_multiplier=-1)
            ident = cst.tile([128, 128], F32, tag="ident")
            nc.vector.tensor_scalar(ident[:], dmat[:], 0, None, OP.is_equal)

            # W2 lhsT (NP, NP): lhsT[c, o] = W2[o, c]
            w2 = cst.tile([NP, NP], BF, tag="w2")
            nc.vector.memset(w2[:], 0.0)
            # [0:81, 0:81]: 1 iff o - c in {0, 1}
            ge0 = tmp.tile([SE, SE], F32, tag="scr0")
            nc.vector.tensor_scalar(ge0[:], dmat[:SE, :SE], 0, None, OP.is_ge)
            le1 = tmp.tile([SE, SE], F32, tag="scr1")
            nc.vector.tensor_scalar(le1[:], dmat[:SE, :SE], 1, None, OP.is_le)
            nc.vector.tensor_mul(_ap(w2, 0, [[NP, SE], [1, SE]]), ge0[:], le1[:])
            # [0:81, 81:120]: lhsT[c, 81+j] = 1 iff c - 2j in {0, 1}
            i2 = cst.tile([SE, NJ], I32, tag="i2")
            nc.gpsimd.iota(i2[:], pattern=[[-2, NJ]], base=0, channel_multiplier=1)
            gA = tmp.tile([SE, NJ], F32, tag="gA")
            nc.vector.tensor_scalar(gA[:], i2[:], 0, None, OP.is_ge)
            gB = tmp.tile([SE, NJ], F32, tag="gB")
            nc.vector.tensor_scalar(gB[:], i2[:], 1, None, OP.is_le)
            nc.vector.tensor_mul(_ap(w2, SE, [[NP, SE], [1, NJ]]), gA[:], gB[:])
            # rows 81:120 built at base partitions, then DMA'd into place
            scrI = cst.tile([NJ, NP], I32, tag="scrI")
            # cols 0:81: 1 iff f - 2j - 3 == 0
            nc.gpsimd.iota(
                _ap(scrI, 0, [[NP, NJ], [1, SE]]),
                pattern=[[1, SE]], base=-3, channel_multiplier=-2,
            )
            # cols 81:120: 1 iff f - j - 1 == 0
            nc.gpsimd.iota(
                _ap(scrI, SE, [[NP, NJ], [1, NJ]]),
                pattern=[[1, NJ]], base=-1, channel_multiplier=-1,
            )
            scrF = cst.tile([NJ, NP], BF, tag="scrF")
            nc.vector.tensor_scalar(scrF[:], scrI[:], 0, None, OP.is_equal)
            nc.sync.dma_start(_ap(w2, SE * NP, [[NP, NJ], [1, NP]]), scrF[:])

            onesl = cst.tile([SE, 1], BF, tag="onesl")
            nc.vector.memset(onesl[:], 1.0)
            onesb = cst.tile([1, NP], BF, tag="onesb")
            nc.vector.memset(onesb[:], 1.0)
            onesbf = cst.tile([1, NP], F32, tag="onesbf")
            nc.vector.memset(onesbf[:], 1.0)

            # ---------- small inputs ----------
            tgs = cst.tile([NL, S], I32, tag="tgs")
            nc.sync.dma_start(tgs[:], tg_ext[:])
            tls = cst.tile([NL, 1], I32, tag="tls")
            nc.sync.dma_start(tls[:], _ap(tl_ext, 0, [[1, NL], [1, 1]]))

            # et (NL, SE) f32: blank-expanded targets; odd slots get labels
            et = cst.tile([NL, SE], F32, tag="et")
            nc.vector.memset(et[:], 0.0)
            nc.vector.tensor_copy(_ap(et, 1, [[SE, NL], [2, S]]), tgs[:])
            # mfree (NL, SE): col s' holds mask at dest s'+2 = (et[s'+2] != et[s'])
            mfree = cst.tile([NL, SE], F32, tag="mfree")
            nc.vector.memset(mfree[:], 0.0)
            nc.vector.tensor_tensor(
                _ap(mfree, 0, [[SE, NL], [1, SE - 2]]),
                _ap(et, 2, [[SE, NL], [1, SE - 2]]),
                _ap(et, 0, [[SE, NL], [1, SE - 2]]),
                OP.not_equal,
            )

            # ---------- transposes to states-on-partitions ----------
            etT_ps = ps1.tile([SE, NL], F32, tag="tp")
            nc.tensor.transpose(etT_ps[:], et[:], ident[:NL, :NL])
            etT_i = cst.tile([SE, NL], I32, tag="etTi")
            nc.vector.tensor_copy(etT_i[:], etT_ps[:])
            # mask page (NP, NL): rows 0..80 = 1, rows 81+j = mask[2j+3];
            # built as (NL, NP) concat in the free axis, then PE-transposed
            mcat = cst.tile([NL, NP], F32, tag="mcat")
            nc.vector.memset(mcat[:], 1.0)
            nc.vector.tensor_copy(
                _ap(mcat, SE, [[NP, NL], [1, NJ]]),
                _ap(mfree, 1, [[SE, NL], [2, NJ]]),
            )
            mpage_ps = ps1.tile([NP, NL], F32, tag="tp")
            nc.tensor.transpose(mpage_ps[:], mcat[:], ident[:NL, :NL])
            mpage = cst.tile([NP, NL], BF, tag="mpage")
            nc.vector.tensor_copy(mpage[:], mpage_ps[:])
            # target lengths -> row (1, NL) f32
            tlf = cst.tile([NL, 1], F32, tag="tlf")
            nc.vector.tensor_copy(tlf[:], tls[:])
            tlT_ps = ps1.tile([1, NL], F32, tag="tp1")
            nc.tensor.transpose(tlT_ps[:], tlf[:], ident[:NL, :NL])
            lrow = cst.tile([1, NL], F32, tag="lrow")
            nc.vector.tensor_copy(lrow[:], tlT_ps[:])
            l2row = cst.tile([1, NL], F32, tag="l2row")
            nc.vector.tensor_scalar(l2row[:], lrow[:], 2.0, None, OP.mult)
            # thr (NP, NL) = 2L broadcast down partitions (via PE)
            thr_ps = ps1.tile([NP, NL], F32, tag="tp")
            nc.tensor.matmul(thr_ps[:], onesbf[:1, :NP], l2row[:], start=True, stop=True)
            thr = cst.tile([NP, NL], F32, tag="thr")
            nc.vector.tensor_copy(thr[:], thr_ps[:])

            # per-row state value: rows 0..80 -> s, rows 81+j -> 2j+1
            siota = cst.tile([SE, 1], I32, tag="siota")
            nc.gpsimd.iota(siota[:], pattern=[[0, 1]], base=0, channel_multiplier=1)
            siof = cst.tile([SE, 1], F32, tag="siof")
            nc.vector.tensor_copy(siof[:], siota[:])
            vfree = cst.tile([1, NP], I32, tag="vfree")
            nc.gpsimd.iota(
                _ap(vfree, 0, [[NP, 1], [1, SE]]),
                pattern=[[1, SE]], base=0, channel_multiplier=0,
            )
            nc.gpsimd.iota(
                _ap(vfree, SE, [[NP, 1], [1, NJ]]),
                pattern=[[2, NJ]], base=1, channel_multiplier=0,
            )
            vfree_f = cst.tile([1, NP], F32, tag="vfreef")
            nc.vector.tensor_copy(vfree_f[:], vfree[:])
            vrow_ps = ps1.tile([NP, 1], F32, tag="tp")
            nc.tensor.transpose(vrow_ps[:], vfree_f[:], ident[:1, :1])
            vrow = cst.tile([NP, 1], F32, tag="vrow")
            nc.vector.tensor_copy(vrow[:], vrow_ps[:])
            # Wm (NP, NL): 1 iff row-state <= 2L
            wm = cst.tile([NP, NL], BF, tag="wm")
            nc.vector.tensor_tensor(
                wm[:], _ap(vrow, 0, [[1, NP], [0, NL]]), thr[:], OP.is_le
            )

            # ---------- gather offsets + indirect gather ----------
            offs = big.tile([SE, T * NL], I32, tag="offs")
            nc.gpsimd.iota(
                _ap(offs, 0, [[T * NL, SE], [NL, T], [1, NL]]),
                pattern=[[NL * C, T], [C, NL]],
                base=0,
                channel_multiplier=0,
            )
            nc.vector.tensor_tensor(
                offs[:],
                offs[:],
                _ap(etT_i, 0, [[NL, SE], [0, T], [1, NL]]),
                OP.add,
            )
            glp = big.tile([SE, T * NL], F32, tag="glp")
            nc.gpsimd.indirect_dma_start(
                glp[:],
                None,
                bass.AP(lp_ext, 0, [[C, T * NL], [1, C]]),
                bass.IndirectOffsetOnAxis(ap=offs[:], axis=1),
            )

            # ---------- P2 (NP, T*NL): per-t page [p(s); p(2j+1)*m'(2j+1)] ----------
            p2 = big.tile([NP, T * NL], BF, tag="p2")
            shiftb = cst.tile([SE, 1], F32, tag="shiftb")
            nc.vector.memset(shiftb[:], SHIFT)
            nc.scalar.activation(p2[:SE, :], glp[:], AF.Exp, bias=shiftb[:])
            # odd alpha rows copied down to partitions 81+j (plain-offset
            # row DMAs; strided-partition APs break dep tracking), then one
            # full-height masked multiply (rows 0..80 scale by 1.0)
            for j in range(NJ):
                nc.sync.dma_start(
                    _ap(p2, (SE + j) * T * NL, [[T * NL, 1], [1, T * NL]]),
                    _ap(p2, (2 * j + 1) * T * NL, [[T * NL, 1], [1, T * NL]]),
                )
            nc.vector.tensor_tensor(
                p2[:],
                p2[:],
                _ap(mpage, 0, [[NL, NP], [0, T], [1, NL]]),
                OP.mult,
            )

            # ---------- scan ----------
            rlog = cst.tile([1, n_ren * NL], F32, tag="rlog")

            x = xpool.tile([NP, NL], BF, tag="X")
            nc.vector.memset(x[:], 0.0)
            nc.vector.tensor_copy(x[:2, :], p2[:2, :NL])
            nc.sync.dma_start(
                _ap(x, SE * NL, [[NL, 1], [1, NL]]),
                _ap(p2, SE * T * NL, [[T * NL, 1], [1, NL]]),
            )

            jren = 0
            H = NL // 2
            for t in range(1, T):
                # two independent sample-chains so PE and DVE overlap
                acc_a = psp.tile([NP, H], F32, tag="pa")
                nc.tensor.matmul(
                    acc_a[:], w2[:], _ap(x, 0, [[NL, NP], [1, H]]),
                    start=True, stop=True,
                )
                acc_b = psp.tile([NP, H], F32, tag="pb")
                nc.tensor.matmul(
                    acc_b[:], w2[:], _ap(x, H, [[NL, NP], [1, H]]),
                    start=True, stop=True,
                )
                xn = xpool.tile([NP, NL], BF, tag="X")
                nc.vector.tensor_tensor(
                    _ap(xn, 0, [[NL, NP], [1, H]]),
                    acc_a[:],
                    _ap(p2, t * NL, [[T * NL, NP], [1, H]]),
                    OP.mult,
                )
                nc.vector.tensor_tensor(
                    _ap(xn, H, [[NL, NP], [1, H]]),
                    acc_b[:],
                    _ap(p2, t * NL + H, [[T * NL, NP], [1, H]]),
                    OP.mult,
                )
                x = xn

                if (t % K_RENORM == 0 and t != T - 1) or t == T - 1:
                    aw = xpool.tile([NP, NL], BF, tag="X")
                    nc.vector.tensor_tensor(aw[:], x[:], wm[:], OP.mult)
                    rs = ps1.tile([1, NL], F32, tag="tp1")
                    nc.tensor.matmul(
                        rs[:], onesl[:], aw[:SE, :], start=True, stop=True
                    )
                    rr = tmp.tile([1, NL], BF, tag="rr")
                    with nc.allow_low_precision(reason="renorm scale; log uses exact rs"):
                        nc.vector.reciprocal(rr[:], rs[:])
                    nc.vector.tensor_copy(
                        _ap(rlog, jren, [[n_ren * NL, 1], [n_ren, NL]]),
                        rs[:],
                    )
                    rb = ps1.tile([NP, NL], F32, tag="tp")
                    nc.tensor.matmul(
                        rb[:], onesb[:1, :NP], rr[:], start=True, stop=True
                    )
                    xs = xpool.tile([NP, NL], BF, tag="X")
                    nc.vector.tensor_tensor(xs[:], aw[:], rb[:], OP.mult)
                    xr = xpool.tile([NP, NL], BF, tag="X")
                    nc.vector.tensor_scalar(xr[:], xs[:], CLAMP, None, OP.min)
                    x = xr
                    jren += 1
            assert jren == n_ren

            # ---------- final extraction ----------
            thrm1 = tmp.tile([SE, NL], F32, tag="thrm1")
            nc.vector.tensor_scalar(thrm1[:], thr[:SE, :], 1.0, None, OP.subtract)
            ge = tmp.tile([SE, NL], F32, tag="ge")
            nc.vector.tensor_tensor(
                ge[:], _ap(siof, 0, [[1, SE], [0, NL]]), thrm1[:], OP.is_ge
            )
            le = tmp.tile([SE, NL], F32, tag="le")
            nc.vector.tensor_tensor(
                le[:], _ap(siof, 0, [[1, SE], [0, NL]]), thr[:SE, :], OP.is_le
            )
            wsel = tmp.tile([SE, NL], BF, tag="wsel")
            nc.vector.tensor_mul(wsel[:], ge[:], le[:])
            sel = tmp.tile([SE, NL], BF, tag="sel")
            nc.vector.tensor_mul(sel[:], x[:SE, :], wsel[:])
            tot = ps1.tile([1, NL], F32, tag="tp1")
            nc.tensor.matmul(tot[:], onesl[:], sel[:], start=True, stop=True)
            ltot = tmp.tile([1, NL], F32, tag="ltot")
            nc.scalar.activation(ltot[:], tot[:], AF.Ln)
            # log of scales, then sum over renorm events (n-major layout)
            lr = tmp.tile([1, n_ren * NL], F32, tag="lr")
            nc.scalar.activation(lr[:], rlog[:], AF.Ln)
            slog = tmp.tile([1, NL], F32, tag="slog")
            nc.vector.tensor_reduce(
                slog[:],
                _ap(lr, 0, [[n_ren * NL, 1], [n_ren, NL], [1, n_ren]]),
                mybir.AxisListType.X,
                OP.add,
            )
            q = tmp.tile([1, NL], F32, tag="q")
            nc.vector.tensor_add(q[:], ltot[:], slog[:])
            # (q - T*SHIFT) * -1 = T*SHIFT - q
            q2 = tmp.tile([1, NL], F32, tag="q2")
            nc.vector.tensor_scalar(q2[:], q[:], float(T) * SHIFT, -1.0, OP.subtract, OP.mult)
            rl = tmp.tile([1, NL], F32, tag="rl")
            nc.vector.reciprocal(rl[:], lrow[:])
            loss = tmp.tile([1, NL], F32, tag="loss")
            nc.vector.tensor_mul(loss[:], q2[:], rl[:])
            nc.sync.dma_start(out_ext[:], loss[:])

    nc.compile()
    return nc


_NC_CACHE = {}


def _get_nc(T=T_FULL):
    if T not in _NC_CACHE:
        _NC_CACHE[T] = build_nc(T)
    return _NC_CACHE[T]


def kernel(log_probs, targets, input_lengths, target_lengths):
    lp = np.ascontiguousarray(np.asarray(log_probs, dtype=np.float32))
    tg = np.ascontiguousarray(np.asarray(targets, dtype=np.int32))
    tl = np.ascontiguousarray(np.asarray(target_lengths, dtype=np.int32))
    T, N, _ = lp.shape
    nc = _get_nc(T)
    in_maps = []
    for i in range(NC_CORES):
        s = slice(i * NL, (i + 1) * NL)
        in_maps.append(
            {
                "log_probs": np.ascontiguousarray(lp[:, s, :]),
                "targets": np.ascontiguousarray(tg[s]),
                "target_lengths": np.ascontiguousarray(tl[s]),
            }
        )
    res = run_bass_kernel_spmd(nc, in_maps, core_ids=list(range(NC_CORES)))
    out = np.concatenate([res.results[i]["out"].reshape(NL) for i in range(NC_CORES)])
    return out.astype(np.float32)


# revision 12
# speedup vs baseline: 1.1465x; 1.1465x over previous
"""CTC loss forward on 8 TRN2 NeuronCores, data-parallel over batch.

Problem: log_probs (512, 32, 8000) f32, targets (32, 40) i32,
target_lengths (32,) i32 -> per-sample loss (32,) f32
(input_lengths is ignored, matching the reference).

Strategy per core (4 samples):
 - Gather only the needed log-prob entries: glp[s, t, n] = lp[t, n, et[n, s]]
   (T*4*81 = 166K elements) via one indirect DMA; the 512MB tensor is
   never streamed.
 - Run the T-step DP in linear probability space with an augmented state
   on partitions: rows 0..80 = alpha over the 81 CTC states, rows
   81..119 = the 39 masked skip terms am[j] = alpha[2j+1]*mask[2j+3].
   One constant 120x120 matmul performs all shifts AND regenerates the
   duplicated skip rows; one FD=4 DVE multiply by the precomputed
   per-step probability page completes the step:
       X' = (W2 @ X) * P2[:, t]
 - Every K=8 steps renormalize by the per-sample sum of alpha over
   states s <= 2L (window excludes padding states that run away),
   clamp, and log-accumulate the scales.
 - loss = -(log(alpha[2L] + alpha[2L-1]) + sum(log scales) - T*SHIFT)/L
"""
import sys

for _p in ("/opt/trn_rl_repo",):
    if _p not in sys.path:
        sys.path.append(_p)

import numpy as np
import concourse.bass as bass
import concourse.bacc as bacc
import concourse.mybir as mybir
from concourse import tile
from concourse.bass_utils import run_bass_kernel_spmd

F32 = mybir.dt.float32
I32 = mybir.dt.int32
BF = mybir.dt.bfloat16
AF = mybir.ActivationFunctionType
OP = mybir.AluOpType

T_FULL = 512
NL = 4          # samples per core
NC_CORES = 8
C = 8000
S = 40
SE = 2 * S + 1  # 81
NJ = 39         # skip rows: odd states 1,3,..,77
NP = SE + NJ    # 120 partitions of augmented state
K_RENORM = 16
SHIFT = 9.0
E_SHIFT = float(np.float32(np.exp(np.float32(SHIFT))))
CLAMP = 1e26


def _ap(t, off, dims):
    a = t[:]
    return bass.AP(a.tensor, off, [list(d) for d in dims])


def build_nc(T=T_FULL):
    nc = bacc.Bacc("TRN2", target_bir_lowering=False, debug=True)
    lp_ext = nc.declare_dram_parameter("log_probs", [T, NL, C], F32, isOutput=False)
    tg_ext = nc.declare_dram_parameter("targets", [NL, S], I32, isOutput=False)
    tl_ext = nc.declare_dram_parameter("target_lengths", [NL], I32, isOutput=False)
    out_ext = nc.declare_dram_parameter("out", [1, NL], F32, isOutput=True)

    n_ren = len(range(K_RENORM, T - K_RENORM + 1, K_RENORM))

    with tile.TileContext(nc) as tc:
        with (
            tc.tile_pool(name="cst", bufs=1) as cst,
            tc.tile_pool(name="big", bufs=1) as big,
            tc.tile_pool(name="x", bufs=3) as xpool,
            tc.tile_pool(name="tmp", bufs=2) as tmp,
            tc.tile_pool(name="ps", bufs=2, space=bass.MemorySpace.PSUM) as psp,
            tc.tile_pool(name="ps1", bufs=2, space=bass.MemorySpace.PSUM) as ps1,
        ):
            # ---------- constants built on device ----------
            dmat = cst.tile([128, 128], I32, tag="dmat")
            nc.gpsimd.iota(dmat[:], pattern=[[1, 128]], base=0, channel_multiplier=-1)
            ident = cst.tile([128, 128], F32, tag="ident")
            nc.vector.tensor_scalar(ident[:], dmat[:], 0, None, OP.is_equal)

            # W2 lhsT (NP, NP): lhsT[c, o] = W2[o, c]
            w2 = cst.tile([NP, NP], BF, tag="w2")
            nc.vector.memset(w2[:], 0.0)
            # [0:81, 0:81]: 1 iff o - c in {0, 1}
            ge0 = tmp.tile([SE, SE], F32, tag="scr0")
            nc.vector.tensor_scalar(ge0[:], dmat[:SE, :SE], 0, None, OP.is_ge)
            le1 = tmp.tile([SE, SE], F32, tag="scr1")
            nc.vector.tensor_scalar(le1[:], dmat[:SE, :SE], 1, None, OP.is_le)
            nc.vector.tensor_mul(_ap(w2, 0, [[NP, SE], [1, SE]]), ge0[:], le1[:])
            # [0:81, 81:120]: lhsT[c, 81+j] = 1 iff c - 2j in {0, 1}
            i2 = cst.tile([SE, NJ], I32, tag="i2")
            nc.gpsimd.iota(i2[:], pattern=[[-2, NJ]], base=0, channel_multiplier=1)
            gA = tmp.tile([SE, NJ], F32, tag="gA")
            nc.vector.tensor_scalar(gA[:], i2[:], 0, None, OP.is_ge)
            gB = tmp.tile([SE, NJ], F32, tag="gB")
            nc.vector.tensor_scalar(gB[:], i2[:], 1, None, OP.is_le)
            nc.vector.tensor_mul(_ap(w2, SE, [[NP, SE], [1, NJ]]), gA[:], gB[:])
            # rows 81:120 built at base partitions, then DMA'd into place
            scrI = cst.tile([NJ, NP], I32, tag="scrI")
            # cols 0:81: 1 iff f - 2j - 3 == 0
            nc.gpsimd.iota(
                _ap(scrI, 0, [[NP, NJ], [1, SE]]),
                pattern=[[1, SE]], base=-3, channel_multiplier=-2,
            )
            # cols 81:120: 1 iff f - j - 1 == 0
            nc.gpsimd.iota(
                _ap(scrI, SE, [[NP, NJ], [1, NJ]]),
                pattern=[[1, NJ]], base=-1, channel_multiplier=-1,
            )
            scrF = cst.tile([NJ, NP], BF, tag="scrF")
            nc.vector.tensor_scalar(scrF[:], scrI[:], 0, None, OP.is_equal)
            nc.sync.dma_start(_ap(w2, SE * NP, [[NP, NJ], [1, NP]]), scrF[:])

            onesl = cst.tile([SE, 1], BF, tag="onesl")
            nc.vector.memset(onesl[:], 1.0)
            onesb = cst.tile([1, NP], BF, tag="onesb")
            nc.vector.memset(onesb[:], 1.0)
            onesbf = cst.tile([1, NP], F32, tag="onesbf")
            nc.vector.memset(onesbf[:], 1.0)

            # ---------- small inputs ----------
            tgs = cst.tile([NL, S], I32, tag="tgs")
            nc.sync.dma_start(tgs[:], tg_ext[:])
            tls = cst.tile([NL, 1], I32, tag="tls")
            nc.sync.dma_start(tls[:], _ap(tl_ext, 0, [[1, NL], [1, 1]]))

            # et (NL, SE) f32: blank-expanded targets; odd slots get labels
            et = cst.tile([NL, SE], F32, tag="et")
            nc.vector.memset(et[:], 0.0)
            nc.vector.tensor_copy(_ap(et, 1, [[SE, NL], [2, S]]), tgs[:])
            # mfree (NL, SE): col s' holds mask at dest s'+2 = (et[s'+2] != et[s'])
            mfree = cst.tile([NL, SE], F32, tag="mfree")
            nc.vector.memset(mfree[:], 0.0)
            nc.vector.tensor_tensor(
                _ap(mfree, 0, [[SE, NL], [1, SE - 2]]),
                _ap(et, 2, [[SE, NL], [1, SE - 2]]),
                _ap(et, 0, [[SE, NL], [1, SE - 2]]),
                OP.not_equal,
            )

            # ---------- transposes to states-on-partitions ----------
            etT_ps = ps1.tile([SE, NL], F32, tag="tp")
            nc.tensor.transpose(etT_ps[:], et[:], ident[:NL, :NL])
            etT_i = cst.tile([SE, NL], I32, tag="etTi")
            nc.vector.tensor_copy(etT_i[:], etT_ps[:])
            # mask page (NP, NL): rows 0..80 = 1, rows 81+j = mask[2j+3];
            # built as (NL, NP) concat in the free axis, then PE-transposed
            mcat = cst.tile([NL, NP], F32, tag="mcat")
            nc.vector.memset(mcat[:], 1.0)
            nc.vector.tensor_copy(
                _ap(mcat, SE, [[NP, NL], [1, NJ]]),
                _ap(mfree, 1, [[SE, NL], [2, NJ]]),
            )
            mpage_ps = ps1.tile([NP, NL], F32, tag="tp")
            nc.tensor.transpose(mpage_ps[:], mcat[:], ident[:NL, :NL])
            mpage = cst.tile([NP, NL], BF, tag="mpage")
            nc.vector.tensor_copy(mpage[:], mpage_ps[:])
            # target lengths -> row (1, NL) f32
            tlf = cst.tile([NL, 1], F32, tag="tlf")
            nc.vector.tensor_copy(tlf[:], tls[:])
            tlT_ps = ps1.tile([1, NL], F32, tag="tp1")
            nc.tensor.transpose(tlT_ps[:], tlf[:], ident[:NL, :NL])
            lrow = cst.tile([1, NL], F32, tag="lrow")
            nc.vector.tensor_copy(lrow[:], tlT_ps[:])
            l2row = cst.tile([1, NL], F32, tag="l2row")
            nc.vector.tensor_scalar(l2row[:], lrow[:], 2.0, None, OP.mult)
            # thr (NP, NL) = 2L broadcast down partitions (via PE)
            thr_ps = ps1.tile([NP, NL], F32, tag="tp")
            nc.tensor.matmul(thr_ps[:], onesbf[:1, :NP], l2row[:], start=True, stop=True)
            thr = cst.tile([NP, NL], F32, tag="thr")
            nc.vector.tensor_copy(thr[:], thr_ps[:])

            # per-row state value: rows 0..80 -> s, rows 81+j -> 2j+1
            siota = cst.tile([SE, 1], I32, tag="siota")
            nc.gpsimd.iota(siota[:], pattern=[[0, 1]], base=0, channel_multiplier=1)
            siof = cst.tile([SE, 1], F32, tag="siof")
            nc.vector.tensor_copy(siof[:], siota[:])
            vfree = cst.tile([1, NP], I32, tag="vfree")
            nc.gpsimd.iota(
                _ap(vfree, 0, [[NP, 1], [1, SE]]),
                pattern=[[1, SE]], base=0, channel_multiplier=0,
            )
            nc.gpsimd.iota(
                _ap(vfree, SE, [[NP, 1], [1, NJ]]),
                pattern=[[2, NJ]], base=1, channel_multiplier=0,
            )
            vfree_f = cst.tile([1, NP], F32, tag="vfreef")
            nc.vector.tensor_copy(vfree_f[:], vfree[:])
            vrow_ps = ps1.tile([NP, 1], F32, tag="tp")
            nc.tensor.transpose(vrow_ps[:], vfree_f[:], ident[:1, :1])
            vrow = cst.tile([NP, 1], F32, tag="vrow")
            nc.vector.tensor_copy(vrow[:], vrow_ps[:])
            # Wm (NP, NL): 1 iff row-state <= 2L
            wm = cst.tile([NP, NL], BF, tag="wm")
            nc.vector.tensor_tensor(
                wm[:], _ap(vrow, 0, [[1, NP], [0, NL]]), thr[:], OP.is_le
            )

            # ---------- gather offsets + indirect gather ----------
            offs = big.tile([SE, T * NL], I32, tag="offs")
            nc.gpsimd.iota(
                _ap(offs, 0, [[T * NL, SE], [NL, T], [1, NL]]),
                pattern=[[NL * C, T], [C, NL]],
                base=0,
                channel_multiplier=0,
            )
            nc.vector.tensor_tensor(
                offs[:],
                offs[:],
                _ap(etT_i, 0, [[NL, SE], [0, T], [1, NL]]),
                OP.add,
            )
            glp = big.tile([SE, T * NL], F32, tag="glp")
            nc.gpsimd.indirect_dma_start(
                glp[:],
                None,
                bass.AP(lp_ext, 0, [[C, T * NL], [1, C]]),
                bass.IndirectOffsetOnAxis(ap=offs[:], axis=1),
            )

            # ---------- P2 (NP, T*NL): per-t page [p(s); p(2j+1)*m'(2j+1)] ----------
            p2 = big.tile([NP, T * NL], BF, tag="p2")
            shiftb = cst.tile([SE, 1], F32, tag="shiftb")
            nc.vector.memset(shiftb[:], SHIFT)
            nc.scalar.activation(p2[:SE, :], glp[:], AF.Exp, bias=shiftb[:])
            # odd alpha rows copied down to partitions 81+j (plain-offset
            # row DMAs; strided-partition APs break dep tracking), then one
            # full-height masked multiply (rows 0..80 scale by 1.0)
            for j in range(NJ):
                nc.sync.dma_start(
                    _ap(p2, (SE + j) * T * NL, [[T * NL, 1], [1, T * NL]]),
                    _ap(p2, (2 * j + 1) * T * NL, [[T * NL, 1], [1, T * NL]]),
                )
            nc.vector.tensor_tensor(
                p2[:],
                p2[:],
                _ap(mpage, 0, [[NL, NP], [0, T], [1, NL]]),
                OP.mult,
            )

            # ---------- scan ----------
            rlog = cst.tile([1, n_ren * NL], F32, tag="rlog")

            x = xpool.tile([NP, NL], BF, tag="X")
            nc.vector.memset(x[:], 0.0)
            nc.vector.tensor_copy(x[:2, :], p2[:2, :NL])
            nc.sync.dma_start(
                _ap(x, SE * NL, [[NL, 1], [1, NL]]),
                _ap(p2, SE * T * NL, [[T * NL, 1], [1, NL]]),
            )

            jren = 0
            pend_rr = None
            pend_t0 = -100
            bounds = set(range(K_RENORM, T - K_RENORM + 1, K_RENORM))
            for t in range(1, T):
                acc = psp.tile([NP, NL], F32, tag="pa")
                nc.tensor.matmul(acc[:], w2[:], x[:], start=True, stop=True)
                xn = xpool.tile([NP, NL], BF, tag="X")
                nc.vector.tensor_tensor(
                    xn[:],
                    acc[:],
                    _ap(p2, t * NL, [[T * NL, NP], [1, NL]]),
                    OP.mult,
                )
                x = xn

                # lazy renorm: measure at boundary t0; ops spread over the
                # following idle engine slots; the combined (1/rs)*window
                # scale lands in the probability page consumed at t0+8
                k = t - pend_t0
                if pend_rr is not None:
                    if k == 1:
                        rs = ps1.tile([1, NL], F32, tag="tp1")
                        nc.tensor.matmul(
                            rs[:], onesl[:], pend_rr["aw"][:SE, :],
                            start=True, stop=True,
                        )
                        pend_rr["rs"] = rs
                    elif k == 2:
                        rr = tmp.tile([1, NL], BF, tag="rr")
                        with nc.allow_low_precision(reason="renorm scale"):
                            nc.vector.reciprocal(rr[:], pend_rr["rs"][:])
                        pend_rr["rr"] = rr
                    elif k == 3:
                        rb = ps1.tile([NP, NL], F32, tag="tp")
                        nc.tensor.matmul(
                            rb[:], onesb[:1, :NP], pend_rr["rr"][:],
                            start=True, stop=True,
                        )
                        pend_rr["rb"] = rb
                    elif k == 4:
                        rbw = tmp.tile([NP, NL], BF, tag="rbw")
                        nc.vector.tensor_tensor(
                            rbw[:], pend_rr["rb"][:], wm[:], OP.mult
                        )
                        pend_rr["rbw"] = rbw
                    elif k == 5:
                        page = pend_t0 + 8
                        nc.vector.tensor_tensor(
                            _ap(p2, page * NL, [[T * NL, NP], [1, NL]]),
                            _ap(p2, page * NL, [[T * NL, NP], [1, NL]]),
                            pend_rr["rbw"][:],
                            OP.mult,
                        )
                    elif k == 6:
                        nc.vector.tensor_copy(
                            _ap(rlog, jren, [[n_ren * NL, 1], [n_ren, NL]]),
                            pend_rr["rs"][:],
                        )
                        jren += 1
                        pend_rr = None

                if t in bounds:
                    aw = xpool.tile([NP, NL], BF, tag="AW")
                    nc.vector.tensor_tensor(aw[:], x[:], wm[:], OP.mult)
                    pend_rr = {"aw": aw}
                    pend_t0 = t
            assert jren == n_ren

            # ---------- final extraction ----------
            thrm1 = tmp.tile([SE, NL], F32, tag="thrm1")
            nc.vector.tensor_scalar(thrm1[:], thr[:SE, :], 1.0, None, OP.subtract)
            ge = tmp.tile([SE, NL], F32, tag="ge")
            nc.vector.tensor_tensor(
                ge[:], _ap(siof, 0, [[1, SE], [0, NL]]), thrm1[:], OP.is_ge
            )
            le = tmp.tile([SE, NL], F32, tag="le")
            nc.vector.tensor_tensor(
                le[:], _ap(siof, 0, [[1, SE], [0, NL]]), thr[:SE, :], OP.is_le
            )
            wsel = tmp.tile([SE, NL], BF, tag="wsel")
            nc.vector.tensor_mul(wsel[:], ge[:], le[:])
            sel = tmp.tile([SE, NL], BF, tag="sel")
            nc.vector.tensor_mul(sel[:], x[:SE, :], wsel[:])
            tot = ps1.tile([1, NL], F32, tag="tp1")
            nc.tensor.matmul(tot[:], onesl[:], sel[:], start=True, stop=True)
            ltot = tmp.tile([1, NL], F32, tag="ltot")
            nc.scalar.activation(ltot[:], tot[:], AF.Ln)
            # log of scales, then sum over renorm events (n-major layout)
            lr = tmp.tile([1, n_ren * NL], F32, tag="lr")
            nc.scalar.activation(lr[:], rlog[:], AF.Ln)
            slog = tmp.tile([1, NL], F32, tag="slog")
            nc.vector.tensor_reduce(
                slog[:],
                _ap(lr, 0, [[n_ren * NL, 1], [n_ren, NL], [1, n_ren]]),
                mybir.AxisListType.X,
                OP.add,
            )
            q = tmp.tile([1, NL], F32, tag="q")
            nc.vector.tensor_add(q[:], ltot[:], slog[:])
            # (q - T*SHIFT) * -1 = T*SHIFT - q
            q2 = tmp.tile([1, NL], F32, tag="q2")
            nc.vector.tensor_scalar(q2[:], q[:], float(T) * SHIFT, -1.0, OP.subtract, OP.mult)
            rl = tmp.tile([1, NL], F32, tag="rl")
            nc.vector.reciprocal(rl[:], lrow[:])
            loss = tmp.tile([1, NL], F32, tag="loss")
            nc.vector.tensor_mul(loss[:], q2[:], rl[:])
            nc.sync.dma_start(out_ext[:], loss[:])

    nc.compile()
    return nc


_NC_CACHE = {}


def _get_nc(T=T_FULL):
    if T not in _NC_CACHE:
        _NC_CACHE[T] = build_nc(T)
    return _NC_CACHE[T]


def kernel(log_probs, targets, input_lengths, target_lengths):
    lp = np.ascontiguousarray(np.asarray(log_probs, dtype=np.float32))
    tg = np.ascontiguousarray(np.asarray(targets, dtype=np.int32))
    tl = np.ascontiguousarray(np.asarray(target_lengths, dtype=np.int32))
    T, N, _ = lp.shape
    nc = _get_nc(T)
    in_maps = []
    for i in range(NC_CORES):
        s = slice(i * NL, (i + 1) * NL)
        in_maps.append(
            {
                "log_probs": np.ascontiguousarray(lp[:, s, :]),
                "targets": np.ascontiguousarray(tg[s]),
                "target_lengths": np.ascontiguousarray(tl[s]),
            }
        )
    res = run_bass_kernel_spmd(nc, in_maps, core_ids=list(range(NC_CORES)))
    out = np.concatenate([res.results[i]["out"].reshape(NL) for i in range(NC_CORES)])
    return out.astype(np.float32)


# revision 13
# speedup vs baseline: 1.2610x; 1.0999x over previous
"""CTC loss forward on 8 TRN2 NeuronCores, data-parallel over batch.

Problem: log_probs (512, 32, 8000) f32, targets (32, 40) i32,
target_lengths (32,) i32 -> per-sample loss (32,) f32
(input_lengths is ignored, matching the reference).

Strategy per core (4 samples):
 - Gather only the needed log-prob entries: glp[s, t, n] = lp[t, n, et[n, s]]
   (T*4*81 = 166K elements) via one indirect DMA; the 512MB tensor is
   never streamed.
 - Run the T-step DP in linear probability space with an augmented state
   on partitions: rows 0..80 = alpha over the 81 CTC states, rows
   81..119 = the 39 masked skip terms am[j] = alpha[2j+1]*mask[2j+3].
   One constant 120x120 matmul performs all shifts AND regenerates the
   duplicated skip rows; one FD=4 DVE multiply by the precomputed
   per-step probability page completes the step:
       X' = (W2 @ X) * P2[:, t]
 - Every K=8 steps renormalize by the per-sample sum of alpha over
   states s <= 2L (window excludes padding states that run away),
   clamp, and log-accumulate the scales.
 - loss = -(log(alpha[2L] + alpha[2L-1]) + sum(log scales) - T*SHIFT)/L
"""
import sys

for _p in ("/opt/trn_rl_repo",):
    if _p not in sys.path:
        sys.path.append(_p)

import numpy as np
import concourse.bass as bass
import concourse.bacc as bacc
import concourse.mybir as mybir
from concourse import tile
from concourse.bass_utils import run_bass_kernel_spmd

F32 = mybir.dt.float32
I32 = mybir.dt.int32
BF = mybir.dt.bfloat16
AF = mybir.ActivationFunctionType
OP = mybir.AluOpType

T_FULL = 512
NL = 4          # samples per core
NC_CORES = 8
C = 8000
S = 40
SE = 2 * S + 1  # 81
NJ = 39         # skip rows: odd states 1,3,..,77
NP = SE + NJ    # 120 partitions of augmented state
K_RENORM = 16
SHIFT = 9.0
E_SHIFT = float(np.float32(np.exp(np.float32(SHIFT))))
CLAMP = 1e26


def _ap(t, off, dims):
    a = t[:]
    return bass.AP(a.tensor, off, [list(d) for d in dims])


def build_nc(T=T_FULL):
    nc = bacc.Bacc("TRN2", target_bir_lowering=False, debug=True)
    lp_ext = nc.declare_dram_parameter("log_probs", [T, NL, C], F32, isOutput=False)
    tg_ext = nc.declare_dram_parameter("targets", [NL, S], I32, isOutput=False)
    tl_ext = nc.declare_dram_parameter("target_lengths", [NL], I32, isOutput=False)
    out_ext = nc.declare_dram_parameter("out", [1, NL], F32, isOutput=True)

    n_ren = len(range(K_RENORM, T - K_RENORM + 1, K_RENORM))

    with tile.TileContext(nc) as tc:
        with (
            tc.tile_pool(name="cst", bufs=1) as cst,
            tc.tile_pool(name="big", bufs=1) as big,
            tc.tile_pool(name="x", bufs=3) as xpool,
            tc.tile_pool(name="tmp", bufs=2) as tmp,
            tc.tile_pool(name="ps", bufs=2, space=bass.MemorySpace.PSUM) as psp,
            tc.tile_pool(name="ps1", bufs=2, space=bass.MemorySpace.PSUM) as ps1,
        ):
            # ---------- constants built on device ----------
            dmat = cst.tile([128, 128], I32, tag="dmat")
            nc.gpsimd.iota(dmat[:], pattern=[[1, 128]], base=0, channel_multiplier=-1)
            ident = cst.tile([128, 128], F32, tag="ident")
            nc.vector.tensor_scalar(ident[:], dmat[:], 0, None, OP.is_equal)

            # W2 lhsT (NP, NP): lhsT[c, o] = W2[o, c]
            w2 = cst.tile([NP, NP], BF, tag="w2")
            nc.vector.memset(w2[:], 0.0)
            # [0:81, 0:81]: 1 iff o - c in {0, 1}
            ge0 = tmp.tile([SE, SE], F32, tag="scr0")
            nc.vector.tensor_scalar(ge0[:], dmat[:SE, :SE], 0, None, OP.is_ge)
            le1 = tmp.tile([SE, SE], F32, tag="scr1")
            nc.vector.tensor_scalar(le1[:], dmat[:SE, :SE], 1, None, OP.is_le)
            nc.vector.tensor_mul(_ap(w2, 0, [[NP, SE], [1, SE]]), ge0[:], le1[:])
            # [0:81, 81:120]: lhsT[c, 81+j] = 1 iff c - 2j in {0, 1}
            i2 = cst.tile([SE, NJ], I32, tag="i2")
            nc.gpsimd.iota(i2[:], pattern=[[-2, NJ]], base=0, channel_multiplier=1)
            gA = tmp.tile([SE, NJ], F32, tag="gA")
            nc.vector.tensor_scalar(gA[:], i2[:], 0, None, OP.is_ge)
            gB = tmp.tile([SE, NJ], F32, tag="gB")
            nc.vector.tensor_scalar(gB[:], i2[:], 1, None, OP.is_le)
            nc.vector.tensor_mul(_ap(w2, SE, [[NP, SE], [1, NJ]]), gA[:], gB[:])
            # rows 81:120 built at base partitions, then DMA'd into place
            scrI = cst.tile([NJ, NP], I32, tag="scrI")
            # cols 0:81: 1 iff f - 2j - 3 == 0
            nc.gpsimd.iota(
                _ap(scrI, 0, [[NP, NJ], [1, SE]]),
                pattern=[[1, SE]], base=-3, channel_multiplier=-2,
            )
            # cols 81:120: 1 iff f - j - 1 == 0
            nc.gpsimd.iota(
                _ap(scrI, SE, [[NP, NJ], [1, NJ]]),
                pattern=[[1, NJ]], base=-1, channel_multiplier=-1,
            )
            scrF = cst.tile([NJ, NP], BF, tag="scrF")
            nc.vector.tensor_scalar(scrF[:], scrI[:], 0, None, OP.is_equal)
            nc.sync.dma_start(_ap(w2, SE * NP, [[NP, NJ], [1, NP]]), scrF[:])

            onesl = cst.tile([SE, 1], BF, tag="onesl")
            nc.vector.memset(onesl[:], 1.0)
            onesb = cst.tile([1, NP], BF, tag="onesb")
            nc.vector.memset(onesb[:], 1.0)
            onesbf = cst.tile([1, NP], F32, tag="onesbf")
            nc.vector.memset(onesbf[:], 1.0)

            # ---------- small inputs ----------
            tgs = cst.tile([NL, S], I32, tag="tgs")
            nc.sync.dma_start(tgs[:], tg_ext[:])
            tls = cst.tile([NL, 1], I32, tag="tls")
            nc.sync.dma_start(tls[:], _ap(tl_ext, 0, [[1, NL], [1, 1]]))

            # et (NL, SE) f32: blank-expanded targets; odd slots get labels
            et = cst.tile([NL, SE], F32, tag="et")
            nc.vector.memset(et[:], 0.0)
            nc.vector.tensor_copy(_ap(et, 1, [[SE, NL], [2, S]]), tgs[:])
            # mfree (NL, SE): col s' holds mask at dest s'+2 = (et[s'+2] != et[s'])
            mfree = cst.tile([NL, SE], F32, tag="mfree")
            nc.vector.memset(mfree[:], 0.0)
            nc.vector.tensor_tensor(
                _ap(mfree, 0, [[SE, NL], [1, SE - 2]]),
                _ap(et, 2, [[SE, NL], [1, SE - 2]]),
                _ap(et, 0, [[SE, NL], [1, SE - 2]]),
                OP.not_equal,
            )

            # ---------- transposes to states-on-partitions ----------
            # class ids for all NP rows: [et | labels of odd states]
            etcat = cst.tile([NL, NP], F32, tag="etcat")
            nc.vector.tensor_copy(_ap(etcat, 0, [[NP, NL], [1, SE]]), et[:])
            nc.vector.tensor_copy(
                _ap(etcat, SE, [[NP, NL], [1, NJ]]),
                _ap(tgs, 0, [[S, NL], [1, NJ]]),
            )
            etT_ps = ps1.tile([NP, NL], F32, tag="tp")
            nc.tensor.transpose(etT_ps[:], etcat[:], ident[:NL, :NL])
            etT_i = cst.tile([NP, NL], I32, tag="etTi")
            nc.vector.tensor_copy(etT_i[:], etT_ps[:])
            # mask page (NP, NL): rows 0..80 = 1, rows 81+j = mask[2j+3];
            # built as (NL, NP) concat in the free axis, then PE-transposed
            mcat = cst.tile([NL, NP], F32, tag="mcat")
            nc.vector.memset(mcat[:], 1.0)
            nc.vector.tensor_copy(
                _ap(mcat, SE, [[NP, NL], [1, NJ]]),
                _ap(mfree, 1, [[SE, NL], [2, NJ]]),
            )
            mpage_ps = ps1.tile([NP, NL], F32, tag="tp")
            nc.tensor.transpose(mpage_ps[:], mcat[:], ident[:NL, :NL])
            mpage = cst.tile([NP, NL], BF, tag="mpage")
            nc.vector.tensor_copy(mpage[:], mpage_ps[:])
            # target lengths -> row (1, NL) f32
            tlf = cst.tile([NL, 1], F32, tag="tlf")
            nc.vector.tensor_copy(tlf[:], tls[:])
            tlT_ps = ps1.tile([1, NL], F32, tag="tp1")
            nc.tensor.transpose(tlT_ps[:], tlf[:], ident[:NL, :NL])
            lrow = cst.tile([1, NL], F32, tag="lrow")
            nc.vector.tensor_copy(lrow[:], tlT_ps[:])
            l2row = cst.tile([1, NL], F32, tag="l2row")
            nc.vector.tensor_scalar(l2row[:], lrow[:], 2.0, None, OP.mult)
            # thr (NP, NL) = 2L broadcast down partitions (via PE)
            thr_ps = ps1.tile([NP, NL], F32, tag="tp")
            nc.tensor.matmul(thr_ps[:], onesbf[:1, :NP], l2row[:], start=True, stop=True)
            thr = cst.tile([NP, NL], F32, tag="thr")
            nc.vector.tensor_copy(thr[:], thr_ps[:])

            # per-row state value: rows 0..80 -> s, rows 81+j -> 2j+1
            siota = cst.tile([SE, 1], I32, tag="siota")
            nc.gpsimd.iota(siota[:], pattern=[[0, 1]], base=0, channel_multiplier=1)
            siof = cst.tile([SE, 1], F32, tag="siof")
            nc.vector.tensor_copy(siof[:], siota[:])
            vfree = cst.tile([1, NP], I32, tag="vfree")
            nc.gpsimd.iota(
                _ap(vfree, 0, [[NP, 1], [1, SE]]),
                pattern=[[1, SE]], base=0, channel_multiplier=0,
            )
            nc.gpsimd.iota(
                _ap(vfree, SE, [[NP, 1], [1, NJ]]),
                pattern=[[2, NJ]], base=1, channel_multiplier=0,
            )
            vfree_f = cst.tile([1, NP], F32, tag="vfreef")
            nc.vector.tensor_copy(vfree_f[:], vfree[:])
            vrow_ps = ps1.tile([NP, 1], F32, tag="tp")
            nc.tensor.transpose(vrow_ps[:], vfree_f[:], ident[:1, :1])
            vrow = cst.tile([NP, 1], F32, tag="vrow")
            nc.vector.tensor_copy(vrow[:], vrow_ps[:])
            # Wm (NP, NL): 1 iff row-state <= 2L
            wm = cst.tile([NP, NL], BF, tag="wm")
            nc.vector.tensor_tensor(
                wm[:], _ap(vrow, 0, [[1, NP], [0, NL]]), thr[:], OP.is_le
            )

            # ---------- gather offsets + chunked indirect gather ----------
            offs = big.tile([NP, T * NL], I32, tag="offs")
            nc.gpsimd.iota(
                _ap(offs, 0, [[T * NL, NP], [NL, T], [1, NL]]),
                pattern=[[NL * C, T], [C, NL]],
                base=0,
                channel_multiplier=0,
            )
            nc.vector.tensor_tensor(
                offs[:],
                offs[:],
                _ap(etT_i, 0, [[NL, NP], [0, T], [1, NL]]),
                OP.add,
            )
            # P2 (NP, T*NL): per-t page [p(s); p(2j+1)*m'(2j+1)], built in
            # T-chunks so the scan can start as soon as early pages land
            glp = big.tile([NP, T * NL], F32, tag="glp")
            p2 = big.tile([NP, T * NL], BF, tag="p2")
            shiftb = cst.tile([NP, 1], F32, tag="shiftb")
            nc.vector.memset(shiftb[:], SHIFT)
            NCH = 8
            TCH = T // NCH
            for c in range(NCH):
                lo = c * TCH * NL
                ncol = TCH * NL
                nc.gpsimd.indirect_dma_start(
                    _ap(glp, lo, [[T * NL, NP], [1, ncol]]),
                    None,
                    bass.AP(lp_ext, 0, [[C, T * NL], [1, C]]),
                    bass.IndirectOffsetOnAxis(
                        ap=_ap(offs, lo, [[T * NL, NP], [1, ncol]]), axis=1
                    ),
                )
                nc.scalar.activation(
                    _ap(p2, lo, [[T * NL, NP], [1, ncol]]),
                    _ap(glp, lo, [[T * NL, NP], [1, ncol]]),
                    AF.Exp,
                    bias=shiftb[:],
                )
                nc.vector.tensor_tensor(
                    _ap(p2, lo, [[T * NL, NP], [1, ncol]]),
                    _ap(p2, lo, [[T * NL, NP], [1, ncol]]),
                    _ap(mpage, 0, [[NL, NP], [0, TCH], [1, NL]]),
                    OP.mult,
                )

            # ---------- scan ----------
            rlog = cst.tile([1, n_ren * NL], F32, tag="rlog")

            x = xpool.tile([NP, NL], BF, tag="X")
            nc.vector.memset(x[:], 0.0)
            nc.vector.tensor_copy(x[:2, :], p2[:2, :NL])
            nc.sync.dma_start(
                _ap(x, SE * NL, [[NL, 1], [1, NL]]),
                _ap(p2, SE * T * NL, [[T * NL, 1], [1, NL]]),
            )

            jren = 0
            pend_rr = None
            pend_t0 = -100
            bounds = set(range(K_RENORM, T - K_RENORM + 1, K_RENORM))
            for t in range(1, T):
                acc = psp.tile([NP, NL], F32, tag="pa")
                nc.tensor.matmul(acc[:], w2[:], x[:], start=True, stop=True)
                xn = xpool.tile([NP, NL], BF, tag="X")
                nc.vector.tensor_tensor(
                    xn[:],
                    acc[:],
                    _ap(p2, t * NL, [[T * NL, NP], [1, NL]]),
                    OP.mult,
                )
                x = xn

                # lazy renorm: measure at boundary t0; ops spread over the
                # following idle engine slots; the combined (1/rs)*window
                # scale lands in the probability page consumed at t0+8
                k = t - pend_t0
                if pend_rr is not None:
                    if k == 1:
                        rs = ps1.tile([1, NL], F32, tag="tp1")
                        nc.tensor.matmul(
                            rs[:], onesl[:], pend_rr["aw"][:SE, :],
                            start=True, stop=True,
                        )
                        pend_rr["rs"] = rs
                    elif k == 2:
                        rr = tmp.tile([1, NL], BF, tag="rr")
                        with nc.allow_low_precision(reason="renorm scale"):
                            nc.vector.reciprocal(rr[:], pend_rr["rs"][:])
                        pend_rr["rr"] = rr
                    elif k == 3:
                        rb = ps1.tile([NP, NL], F32, tag="tp")
                        nc.tensor.matmul(
                            rb[:], onesb[:1, :NP], pend_rr["rr"][:],
                            start=True, stop=True,
                        )
                        pend_rr["rb"] = rb
                    elif k == 4:
                        rbw = tmp.tile([NP, NL], BF, tag="rbw")
                        nc.vector.tensor_tensor(
                            rbw[:], pend_rr["rb"][:], wm[:], OP.mult
                        )
                        pend_rr["rbw"] = rbw
                    elif k == 5:
                        page = pend_t0 + 8
                        nc.vector.tensor_tensor(
                            _ap(p2, page * NL, [[T * NL, NP], [1, NL]]),
                            _ap(p2, page * NL, [[T * NL, NP], [1, NL]]),
                            pend_rr["rbw"][:],
                            OP.mult,
                        )
                    elif k == 6:
                        nc.vector.tensor_copy(
                            _ap(rlog, jren, [[n_ren * NL, 1], [n_ren, NL]]),
                            pend_rr["rs"][:],
                        )
                        jren += 1
                        pend_rr = None

                if t in bounds:
                    aw = xpool.tile([NP, NL], BF, tag="AW")
                    nc.vector.tensor_tensor(aw[:], x[:], wm[:], OP.mult)
                    pend_rr = {"aw": aw}
                    pend_t0 = t
            assert jren == n_ren

            # ---------- final extraction ----------
            thrm1 = tmp.tile([SE, NL], F32, tag="thrm1")
            nc.vector.tensor_scalar(thrm1[:], thr[:SE, :], 1.0, None, OP.subtract)
            ge = tmp.tile([SE, NL], F32, tag="ge")
            nc.vector.tensor_tensor(
                ge[:], _ap(siof, 0, [[1, SE], [0, NL]]), thrm1[:], OP.is_ge
            )
            le = tmp.tile([SE, NL], F32, tag="le")
            nc.vector.tensor_tensor(
                le[:], _ap(siof, 0, [[1, SE], [0, NL]]), thr[:SE, :], OP.is_le
            )
            wsel = tmp.tile([SE, NL], BF, tag="wsel")
            nc.vector.tensor_mul(wsel[:], ge[:], le[:])
            sel = tmp.tile([SE, NL], BF, tag="sel")
            nc.vector.tensor_mul(sel[:], x[:SE, :], wsel[:])
            tot = ps1.tile([1, NL], F32, tag="tp1")
            nc.tensor.matmul(tot[:], onesl[:], sel[:], start=True, stop=True)
            ltot = tmp.tile([1, NL], F32, tag="ltot")
            nc.scalar.activation(ltot[:], tot[:], AF.Ln)
            # log of scales, then sum over renorm events (n-major layout)
            lr = tmp.tile([1, n_ren * NL], F32, tag="lr")
            nc.scalar.activation(lr[:], rlog[:], AF.Ln)
            slog = tmp.tile([1, NL], F32, tag="slog")
            nc.vector.tensor_reduce(
                slog[:],
                _ap(lr, 0, [[n_ren * NL, 1], [n_ren, NL], [1, n_ren]]),
                mybir.AxisListType.X,
                OP.add,
            )
            q = tmp.tile([1, NL], F32, tag="q")
            nc.vector.tensor_add(q[:], ltot[:], slog[:])
            # (q - T*SHIFT) * -1 = T*SHIFT - q
            q2 = tmp.tile([1, NL], F32, tag="q2")
            nc.vector.tensor_scalar(q2[:], q[:], float(T) * SHIFT, -1.0, OP.subtract, OP.mult)
            rl = tmp.tile([1, NL], F32, tag="rl")
            nc.vector.reciprocal(rl[:], lrow[:])
            loss = tmp.tile([1, NL], F32, tag="loss")
            nc.vector.tensor_mul(loss[:], q2[:], rl[:])
            nc.sync.dma_start(out_ext[:], loss[:])

    nc.compile()
    return nc


_NC_CACHE = {}


def _get_nc(T=T_FULL):
    if T not in _NC_CACHE:
        _NC_CACHE[T] = build_nc(T)
    return _NC_CACHE[T]


def kernel(log_probs, targets, input_lengths, target_lengths):
    lp = np.ascontiguousarray(np.asarray(log_probs, dtype=np.float32))
    tg = np.ascontiguousarray(np.asarray(targets, dtype=np.int32))
    tl = np.ascontiguousarray(np.asarray(target_lengths, dtype=np.int32))
    T, N, _ = lp.shape
    nc = _get_nc(T)
    in_maps = []
    for i in range(NC_CORES):
        s = slice(i * NL, (i + 1) * NL)
        in_maps.append(
            {
                "log_probs": np.ascontiguousarray(lp[:, s, :]),
                "targets": np.ascontiguousarray(tg[s]),
                "target_lengths": np.ascontiguousarray(tl[s]),
            }
        )
    res = run_bass_kernel_spmd(nc, in_maps, core_ids=list(range(NC_CORES)))
    out = np.concatenate([res.results[i]["out"].reshape(NL) for i in range(NC_CORES)])
    return out.astype(np.float32)


# revision 16
# speedup vs baseline: 2.0185x; 1.6007x over previous
"""CTC loss forward on 8 TRN2 NeuronCores, data-parallel over batch.

Problem: log_probs (512, 32, 8000) f32, targets (32, 40) i32,
target_lengths (32,) i32 -> per-sample loss (32,) f32
(input_lengths is ignored, matching the reference).

Strategy per core (4 samples):
 - Gather only the needed log-prob entries: glp[s, t, n] = lp[t, n, et[n, s]]
   (T*4*81 = 166K elements) via one indirect DMA; the 512MB tensor is
   never streamed.
 - Run the T-step DP in linear probability space with an augmented state
   on partitions: rows 0..80 = alpha over the 81 CTC states, rows
   81..119 = the 39 masked skip terms am[j] = alpha[2j+1]*mask[2j+3].
   One constant 120x120 matmul performs all shifts AND regenerates the
   duplicated skip rows; one FD=4 DVE multiply by the precomputed
   per-step probability page completes the step:
       X' = (W2 @ X) * P2[:, t]
 - Every K=8 steps renormalize by the per-sample sum of alpha over
   states s <= 2L (window excludes padding states that run away),
   clamp, and log-accumulate the scales.
 - loss = -(log(alpha[2L] + alpha[2L-1]) + sum(log scales) - T*SHIFT)/L
"""
import sys

for _p in ("/opt/trn_rl_repo",):
    if _p not in sys.path:
        sys.path.append(_p)

import numpy as np
import concourse.bass as bass
import concourse.bacc as bacc
import concourse.mybir as mybir
from concourse import tile
from concourse.bass_utils import run_bass_kernel_spmd

F32 = mybir.dt.float32
I32 = mybir.dt.int32
BF = mybir.dt.bfloat16
AF = mybir.ActivationFunctionType
OP = mybir.AluOpType

T_FULL = 512
NL = 4          # samples per core
NC_CORES = 8
C = 8000
S = 40
SE = 2 * S + 1  # 81
NJ = 39         # skip rows: odd states 1,3,..,77
NP = SE + NJ    # 120 partitions of augmented state
K_RENORM = 16
SHIFT = 9.0
E_SHIFT = float(np.float32(np.exp(np.float32(SHIFT))))
CLAMP = 1e26


def _ap(t, off, dims):
    a = t[:]
    return bass.AP(a.tensor, off, [list(d) for d in dims])


def build_nc(T=T_FULL):
    nc = bacc.Bacc("TRN2", target_bir_lowering=False, debug=True)
    lp_ext = nc.declare_dram_parameter("log_probs", [T, NL, C], F32, isOutput=False)
    tg_ext = nc.declare_dram_parameter("targets", [NL, S], I32, isOutput=False)
    tl_ext = nc.declare_dram_parameter("target_lengths", [NL], I32, isOutput=False)
    out_ext = nc.declare_dram_parameter("out", [1, NL], F32, isOutput=True)

    tm_ = T // 2
    n_ren = len(range(K_RENORM, tm_ - K_RENORM + 1, K_RENORM)) + len(
        range(T - K_RENORM, tm_ + K_RENORM - 1, -K_RENORM))

    with tile.TileContext(nc) as tc:
        with (
            tc.tile_pool(name="cst", bufs=1) as cst,
            tc.tile_pool(name="big", bufs=1) as big,
            tc.tile_pool(name="x", bufs=3) as xpool,
            tc.tile_pool(name="tmp", bufs=2) as tmp,
            tc.tile_pool(name="ps", bufs=2, space=bass.MemorySpace.PSUM) as psp,
            tc.tile_pool(name="ps1", bufs=2, space=bass.MemorySpace.PSUM) as ps1,
        ):
            # ---------- constants built on device ----------
            dmat = cst.tile([128, 128], I32, tag="dmat")
            nc.gpsimd.iota(dmat[:], pattern=[[1, 128]], base=0, channel_multiplier=-1)
            ident = cst.tile([128, 128], F32, tag="ident")
            nc.vector.tensor_scalar(ident[:], dmat[:], 0, None, OP.is_equal)

            # W2 lhsT (NP, NP): lhsT[c, o] = W2[o, c]
            w2 = cst.tile([NP, NP], BF, tag="w2")
            nc.vector.memset(w2[:], 0.0)
            # [0:81, 0:81]: 1 iff o - c in {0, 1}
            ge0 = tmp.tile([SE, SE], F32, tag="scr0")
            nc.vector.tensor_scalar(ge0[:], dmat[:SE, :SE], 0, None, OP.is_ge)
            le1 = tmp.tile([SE, SE], F32, tag="scr1")
            nc.vector.tensor_scalar(le1[:], dmat[:SE, :SE], 1, None, OP.is_le)
            nc.vector.tensor_mul(_ap(w2, 0, [[NP, SE], [1, SE]]), ge0[:], le1[:])
            # [0:81, 81:120]: lhsT[c, 81+j] = 1 iff c - 2j in {0, 1}
            i2 = cst.tile([SE, NJ], I32, tag="i2")
            nc.gpsimd.iota(i2[:], pattern=[[-2, NJ]], base=0, channel_multiplier=1)
            gA = tmp.tile([SE, NJ], F32, tag="gA")
            nc.vector.tensor_scalar(gA[:], i2[:], 0, None, OP.is_ge)
            gB = tmp.tile([SE, NJ], F32, tag="gB")
            nc.vector.tensor_scalar(gB[:], i2[:], 1, None, OP.is_le)
            nc.vector.tensor_mul(_ap(w2, SE, [[NP, SE], [1, NJ]]), gA[:], gB[:])
            # rows 81:120 built at base partitions, then DMA'd into place
            scrI = cst.tile([NJ, NP], I32, tag="scrI")
            # cols 0:81: 1 iff f - 2j - 3 == 0
            nc.gpsimd.iota(
                _ap(scrI, 0, [[NP, NJ], [1, SE]]),
                pattern=[[1, SE]], base=-3, channel_multiplier=-2,
            )
            # cols 81:120: 1 iff f - j - 1 == 0
            nc.gpsimd.iota(
                _ap(scrI, SE, [[NP, NJ], [1, NJ]]),
                pattern=[[1, NJ]], base=-1, channel_multiplier=-1,
            )
            scrF = cst.tile([NJ, NP], BF, tag="scrF")
            nc.vector.tensor_scalar(scrF[:], scrI[:], 0, None, OP.is_equal)
            nc.sync.dma_start(_ap(w2, SE * NP, [[NP, NJ], [1, NP]]), scrF[:])

            # W2^T lhsT (for the backward chain): lhsT_b[c, o] = W2[c, o]
            w2t = cst.tile([NP, NP], BF, tag="w2t")
            nc.vector.memset(w2t[:], 0.0)
            # [0:81, 0:81]: 1 iff c - o in {0, 1}  <=>  dmat in {-1, 0}
            geM1 = tmp.tile([SE, SE], F32, tag="scr0")
            nc.vector.tensor_scalar(geM1[:], dmat[:SE, :SE], -1, None, OP.is_ge)
            le0 = tmp.tile([SE, SE], F32, tag="scr1")
            nc.vector.tensor_scalar(le0[:], dmat[:SE, :SE], 0, None, OP.is_le)
            nc.vector.tensor_mul(_ap(w2t, 0, [[NP, SE], [1, SE]]), geM1[:], le0[:])
            # [0:81, 81:120]: 1 iff c - 2j - 3 == 0
            i3 = cst.tile([SE, NJ], I32, tag="i3")
            nc.gpsimd.iota(i3[:], pattern=[[-2, NJ]], base=-3, channel_multiplier=1)
            g3 = tmp.tile([SE, NJ], F32, tag="gA")
            nc.vector.tensor_scalar(g3[:], i3[:], 0, None, OP.is_equal)
            nc.vector.tensor_copy(_ap(w2t, SE, [[NP, SE], [1, NJ]]), g3[:])
            # rows 81:120 built at base partitions, then DMA'd into place
            scrI2 = cst.tile([NJ, NP], I32, tag="scrI2")
            # cols 0:81: 1 iff f - 2j in {0, 1}
            nc.gpsimd.iota(
                _ap(scrI2, 0, [[NP, NJ], [1, SE]]),
                pattern=[[1, SE]], base=0, channel_multiplier=-2,
            )
            # cols 81:120: 1 iff j - f - 1 == 0
            nc.gpsimd.iota(
                _ap(scrI2, SE, [[NP, NJ], [1, NJ]]),
                pattern=[[-1, NJ]], base=-1, channel_multiplier=1,
            )
            scrG = tmp.tile([NJ, SE], F32, tag="scrG")
            nc.vector.tensor_scalar(
                scrG[:], _ap(scrI2, 0, [[NP, NJ], [1, SE]]), 0, None, OP.is_ge
            )
            scrG2 = tmp.tile([NJ, SE], F32, tag="scrG2")
            nc.vector.tensor_scalar(
                scrG2[:], _ap(scrI2, 0, [[NP, NJ], [1, SE]]), 1, None, OP.is_le
            )
            scrF2 = cst.tile([NJ, NP], BF, tag="scrF2")
            nc.vector.tensor_mul(
                _ap(scrF2, 0, [[NP, NJ], [1, SE]]), scrG[:], scrG2[:]
            )
            nc.vector.tensor_scalar(
                _ap(scrF2, SE, [[NP, NJ], [1, NJ]]),
                _ap(scrI2, SE, [[NP, NJ], [1, NJ]]),
                0, None, OP.is_equal,
            )
            nc.sync.dma_start(_ap(w2t, SE * NP, [[NP, NJ], [1, NP]]), scrF2[:])

            onesl = cst.tile([SE, 1], BF, tag="onesl")
            nc.vector.memset(onesl[:], 1.0)
            onesb = cst.tile([1, NP], BF, tag="onesb")
            nc.vector.memset(onesb[:], 1.0)
            onesbf = cst.tile([1, NP], F32, tag="onesbf")
            nc.vector.memset(onesbf[:], 1.0)

            # ---------- small inputs ----------
            tgs = cst.tile([NL, S], I32, tag="tgs")
            nc.sync.dma_start(tgs[:], tg_ext[:])
            tls = cst.tile([NL, 1], I32, tag="tls")
            nc.sync.dma_start(tls[:], _ap(tl_ext, 0, [[1, NL], [1, 1]]))

            # et (NL, SE) f32: blank-expanded targets; odd slots get labels
            et = cst.tile([NL, SE], F32, tag="et")
            nc.vector.memset(et[:], 0.0)
            nc.vector.tensor_copy(_ap(et, 1, [[SE, NL], [2, S]]), tgs[:])
            # mfree (NL, SE): col s' holds mask at dest s'+2 = (et[s'+2] != et[s'])
            mfree = cst.tile([NL, SE], F32, tag="mfree")
            nc.vector.memset(mfree[:], 0.0)
            nc.vector.tensor_tensor(
                _ap(mfree, 0, [[SE, NL], [1, SE - 2]]),
                _ap(et, 2, [[SE, NL], [1, SE - 2]]),
                _ap(et, 0, [[SE, NL], [1, SE - 2]]),
                OP.not_equal,
            )

            # ---------- transposes to states-on-partitions ----------
            # class ids for all NP rows: [et | labels of odd states]
            etcat = cst.tile([NL, NP], F32, tag="etcat")
            nc.vector.tensor_copy(_ap(etcat, 0, [[NP, NL], [1, SE]]), et[:])
            nc.vector.tensor_copy(
                _ap(etcat, SE, [[NP, NL], [1, NJ]]),
                _ap(tgs, 0, [[S, NL], [1, NJ]]),
            )
            etT_ps = ps1.tile([NP, NL], F32, tag="tp")
            nc.tensor.transpose(etT_ps[:], etcat[:], ident[:NL, :NL])
            etT_i = cst.tile([NP, NL], I32, tag="etTi")
            nc.vector.tensor_copy(etT_i[:], etT_ps[:])
            # mask page (NP, NL): rows 0..80 = 1, rows 81+j = mask[2j+3];
            # built as (NL, NP) concat in the free axis, then PE-transposed
            mcat = cst.tile([NL, NP], F32, tag="mcat")
            nc.vector.memset(mcat[:], 1.0)
            nc.vector.tensor_copy(
                _ap(mcat, SE, [[NP, NL], [1, NJ]]),
                _ap(mfree, 1, [[SE, NL], [2, NJ]]),
            )
            mpage_ps = ps1.tile([NP, NL], F32, tag="tp")
            nc.tensor.transpose(mpage_ps[:], mcat[:], ident[:NL, :NL])
            mpage = cst.tile([NP, NL], BF, tag="mpage")
            nc.vector.tensor_copy(mpage[:], mpage_ps[:])
            # target lengths -> row (1, NL) f32
            tlf = cst.tile([NL, 1], F32, tag="tlf")
            nc.vector.tensor_copy(tlf[:], tls[:])
            tlT_ps = ps1.tile([1, NL], F32, tag="tp1")
            nc.tensor.transpose(tlT_ps[:], tlf[:], ident[:NL, :NL])
            lrow = cst.tile([1, NL], F32, tag="lrow")
            nc.vector.tensor_copy(lrow[:], tlT_ps[:])
            l2row = cst.tile([1, NL], F32, tag="l2row")
            nc.vector.tensor_scalar(l2row[:], lrow[:], 2.0, None, OP.mult)
            # thr (NP, NL) = 2L broadcast down partitions (via PE)
            thr_ps = ps1.tile([NP, NL], F32, tag="tp")
            nc.tensor.matmul(thr_ps[:], onesbf[:1, :NP], l2row[:], start=True, stop=True)
            thr = cst.tile([NP, NL], F32, tag="thr")
            nc.vector.tensor_copy(thr[:], thr_ps[:])

            # per-row state value: rows 0..80 -> s, rows 81+j -> 2j+1
            siota = cst.tile([SE, 1], I32, tag="siota")
            nc.gpsimd.iota(siota[:], pattern=[[0, 1]], base=0, channel_multiplier=1)
            siof = cst.tile([SE, 1], F32, tag="siof")
            nc.vector.tensor_copy(siof[:], siota[:])
            vfree = cst.tile([1, NP], I32, tag="vfree")
            nc.gpsimd.iota(
                _ap(vfree, 0, [[NP, 1], [1, SE]]),
                pattern=[[1, SE]], base=0, channel_multiplier=0,
            )
            nc.gpsimd.iota(
                _ap(vfree, SE, [[NP, 1], [1, NJ]]),
                pattern=[[2, NJ]], base=1, channel_multiplier=0,
            )
            vfree_f = cst.tile([1, NP], F32, tag="vfreef")
            nc.vector.tensor_copy(vfree_f[:], vfree[:])
            vrow_ps = ps1.tile([NP, 1], F32, tag="tp")
            nc.tensor.transpose(vrow_ps[:], vfree_f[:], ident[:1, :1])
            vrow = cst.tile([NP, 1], F32, tag="vrow")
            nc.vector.tensor_copy(vrow[:], vrow_ps[:])
            # Wm (NP, NL): 1 iff row-state <= 2L
            wm = cst.tile([NP, NL], BF, tag="wm")
            nc.vector.tensor_tensor(
                wm[:], _ap(vrow, 0, [[1, NP], [0, NL]]), thr[:], OP.is_le
            )

            # ---------- gather offsets + chunked indirect gather ----------
            offs = big.tile([NP, T * NL], I32, tag="offs")
            nc.gpsimd.iota(
                _ap(offs, 0, [[T * NL, NP], [NL, T], [1, NL]]),
                pattern=[[NL * C, T], [C, NL]],
                base=0,
                channel_multiplier=0,
            )
            nc.vector.tensor_tensor(
                offs[:],
                offs[:],
                _ap(etT_i, 0, [[NL, NP], [0, T], [1, NL]]),
                OP.add,
            )
            # P2 (NP, T*NL): per-t page [p(s); p(2j+1)*m'(2j+1)], built in
            # T-chunks so the scan can start as soon as early pages land
            glp = big.tile([NP, T * NL], F32, tag="glp")
            p2 = big.tile([NP, T * NL], BF, tag="p2")
            shiftb = cst.tile([NP, 1], F32, tag="shiftb")
            nc.vector.memset(shiftb[:], SHIFT)
            NCH = 8
            TCH = T // NCH
            chunk_order = []
            a, b = 0, NCH - 1
            while a <= b:
                if b > a:
                    chunk_order += [b, a]
                else:
                    chunk_order += [a]
                a, b = a + 1, b - 1
            for c in chunk_order:
                lo = c * TCH * NL
                ncol = TCH * NL
                nc.gpsimd.indirect_dma_start(
                    _ap(glp, lo, [[T * NL, NP], [1, ncol]]),
                    None,
                    bass.AP(lp_ext, 0, [[C, T * NL], [1, C]]),
                    bass.IndirectOffsetOnAxis(
                        ap=_ap(offs, lo, [[T * NL, NP], [1, ncol]]), axis=1
                    ),
                )
                nc.scalar.activation(
                    _ap(p2, lo, [[T * NL, NP], [1, ncol]]),
                    _ap(glp, lo, [[T * NL, NP], [1, ncol]]),
                    AF.Exp,
                    bias=shiftb[:],
                )
                nc.vector.tensor_tensor(
                    _ap(p2, lo, [[T * NL, NP], [1, ncol]]),
                    _ap(p2, lo, [[T * NL, NP], [1, ncol]]),
                    _ap(mpage, 0, [[NL, NP], [0, TCH], [1, NL]]),
                    OP.mult,
                )

            # ---------- scan: forward and backward chains interleaved ----------
            tm = T // 2
            rlog = cst.tile([1, n_ren * NL], F32, tag="rlog")

            # g init = indicator of states {2L-1, 2L} (suffix extraction vec)
            thrm1 = tmp.tile([SE, NL], F32, tag="thrm1")
            nc.vector.tensor_scalar(thrm1[:], thr[:SE, :], 1.0, None, OP.subtract)
            ge = tmp.tile([SE, NL], F32, tag="ge")
            nc.vector.tensor_tensor(
                ge[:], _ap(siof, 0, [[1, SE], [0, NL]]), thrm1[:], OP.is_ge
            )
            le = tmp.tile([SE, NL], F32, tag="le")
            nc.vector.tensor_tensor(
                le[:], _ap(siof, 0, [[1, SE], [0, NL]]), thr[:SE, :], OP.is_le
            )
            wsel = tmp.tile([SE, NL], BF, tag="wsel")
            nc.vector.tensor_mul(wsel[:], ge[:], le[:])

            x = xpool.tile([NP, NL], BF, tag="X")
            nc.vector.memset(x[:], 0.0)
            nc.vector.tensor_copy(x[:2, :], p2[:2, :NL])
            nc.sync.dma_start(
                _ap(x, SE * NL, [[NL, 1], [1, NL]]),
                _ap(p2, SE * T * NL, [[T * NL, 1], [1, NL]]),
            )
            gx = xpool.tile([NP, NL], BF, tag="G")
            nc.vector.memset(gx[:], 0.0)
            nc.vector.tensor_copy(gx[:SE, :], wsel[:])
            g_is_psum = False

            jren_f = 0
            jren_b = n_ren // 2
            pf = None
            pf_t0 = -100
            pb = None
            pb_t0 = 10 ** 9
            bounds_f = set(range(K_RENORM, tm - K_RENORM + 1, K_RENORM))
            bounds_b = set(range(T - K_RENORM, tm + K_RENORM - 1, -K_RENORM))

            def fwd_renorm_tick(t):
                nonlocal pf, jren_f
                k = t - pf_t0
                if pf is None:
                    return
                if k == 1:
                    rs = ps1.tile([1, NL], F32, tag="tp1")
                    nc.tensor.matmul(
                        rs[:], onesl[:], pf["aw"][:SE, :], start=True, stop=True
                    )
                    pf["rs"] = rs
                elif k == 2:
                    rr = tmp.tile([1, NL], BF, tag="rr")
                    with nc.allow_low_precision(reason="renorm scale"):
                        nc.vector.reciprocal(rr[:], pf["rs"][:])
                    pf["rr"] = rr
                elif k == 3:
                    rb = ps1.tile([NP, NL], F32, tag="tp")
                    nc.tensor.matmul(
                        rb[:], onesb[:1, :NP], pf["rr"][:], start=True, stop=True
                    )
                    pf["rb"] = rb
                elif k == 4:
                    rbw = tmp.tile([NP, NL], BF, tag="rbw")
                    nc.vector.tensor_tensor(rbw[:], pf["rb"][:], wm[:], OP.mult)
                    pf["rbw"] = rbw
                elif k == 5:
                    page = pf_t0 + 8
                    nc.vector.tensor_tensor(
                        _ap(p2, page * NL, [[T * NL, NP], [1, NL]]),
                        _ap(p2, page * NL, [[T * NL, NP], [1, NL]]),
                        pf["rbw"][:],
                        OP.mult,
                    )
                elif k == 6:
                    nc.vector.tensor_copy(
                        _ap(rlog, jren_f, [[n_ren * NL, 1], [n_ren, NL]]),
                        pf["rs"][:],
                    )
                    jren_f += 1
                    pf = None

            def bwd_renorm_tick(t):
                nonlocal pb, jren_b
                k = pb_t0 - t
                if pb is None:
                    return
                if k == 1:
                    rs = ps1.tile([1, NL], F32, tag="tp1")
                    nc.tensor.matmul(
                        rs[:], onesl[:], pb["u"][:SE, :], start=True, stop=True
                    )
                    pb["rs"] = rs
                elif k == 2:
                    rr = tmp.tile([1, NL], BF, tag="rrb")
                    with nc.allow_low_precision(reason="renorm scale"):
                        nc.vector.reciprocal(rr[:], pb["rs"][:])
                    pb["rr"] = rr
                elif k == 3:
                    rb = ps1.tile([NP, NL], F32, tag="tp")
                    nc.tensor.matmul(
                        rb[:], onesb[:1, :NP], pb["rr"][:], start=True, stop=True
                    )
                    pb["rb"] = rb
                elif k == 4:
                    rbw = tmp.tile([NP, NL], BF, tag="rbwb")
                    nc.vector.tensor_copy(rbw[:], pb["rb"][:])
                    pb["rbw"] = rbw
                elif k == 5:
                    page = pb_t0 - 8
                    nc.vector.tensor_tensor(
                        _ap(p2, page * NL, [[T * NL, NP], [1, NL]]),
                        _ap(p2, page * NL, [[T * NL, NP], [1, NL]]),
                        pb["rbw"][:],
                        OP.mult,
                    )
                elif k == 6:
                    nc.vector.tensor_copy(
                        _ap(rlog, jren_b, [[n_ren * NL, 1], [n_ren, NL]]),
                        pb["rs"][:],
                    )
                    jren_b += 1
                    pb = None

            tb = T - 1
            for tf in range(1, tm + 1):
                # forward step tf
                acc = psp.tile([NP, NL], F32, tag="pa")
                nc.tensor.matmul(acc[:], w2[:], x[:], start=True, stop=True)
                xn = xpool.tile([NP, NL], BF, tag="X")
                nc.vector.tensor_tensor(
                    xn[:], acc[:], _ap(p2, tf * NL, [[T * NL, NP], [1, NL]]),
                    OP.mult,
                )
                x = xn
                fwd_renorm_tick(tf)
                if tf in bounds_f:
                    aw = xpool.tile([NP, NL], BF, tag="AW")
                    nc.vector.tensor_tensor(aw[:], x[:], wm[:], OP.mult)
                    pf = {"aw": aw}
                    pf_t0 = tf

                # backward step tb (g_{tb-1} = W2^T (g_tb * P_tb))
                if tb > tm:
                    u = xpool.tile([NP, NL], BF, tag="U")
                    nc.vector.tensor_tensor(
                        u[:], gx[:], _ap(p2, tb * NL, [[T * NL, NP], [1, NL]]),
                        OP.mult,
                    )
                    gacc = psp.tile([NP, NL], F32, tag="pb")
                    nc.tensor.matmul(gacc[:], w2t[:], u[:], start=True, stop=True)
                    gx = gacc
                    bwd_renorm_tick(tb)
                    if tb in bounds_b:
                        pb = {"u": u}
                        pb_t0 = tb
                    tb -= 1
            assert tb == tm
            assert jren_f == n_ren // 2 and jren_b == n_ren

            # ---------- join: loss = -lse(ln a_tm + ln g_tm) ... ----------
            TINY = 1e-37
            xc = tmp.tile([NP, NL], F32, tag="xc")
            nc.vector.tensor_scalar(xc[:], x[:], TINY, None, OP.max)
            la = tmp.tile([NP, NL], F32, tag="la")
            nc.scalar.activation(la[:], xc[:], AF.Ln)
            gc = tmp.tile([NP, NL], F32, tag="gc")
            nc.vector.tensor_scalar(gc[:], gx[:], TINY, None, OP.max)
            lg = tmp.tile([NP, NL], F32, tag="lg")
            nc.scalar.activation(lg[:], gc[:], AF.Ln)
            h0 = tmp.tile([NP, NL], F32, tag="h0")
            nc.vector.tensor_add(h0[:], la[:], lg[:])
            # exclude pairs where either factor flushed to zero:
            # (x <= 0) * -1e9 as an additive penalty
            pa = tmp.tile([NP, NL], F32, tag="pa2")
            nc.vector.tensor_scalar(pa[:], x[:], 0.0, -1e9, OP.is_le, OP.mult)
            pg = tmp.tile([NP, NL], F32, tag="pg2")
            nc.vector.tensor_scalar(pg[:], gx[:], 0.0, -1e9, OP.is_le, OP.mult)
            h1 = tmp.tile([NP, NL], F32, tag="h1")
            nc.vector.tensor_add(h1[:], h0[:], pa[:])
            h = tmp.tile([NP, NL], F32, tag="h")
            nc.vector.tensor_add(h[:], h1[:], pg[:])
            hm = tmp.tile([1, NL], F32, tag="hm")
            nc.gpsimd.tensor_reduce(hm[:], h[:], mybir.AxisListType.C, OP.max)
            hmb = ps1.tile([NP, NL], F32, tag="tp")
            nc.tensor.matmul(hmb[:], onesbf[:1, :NP], hm[:], start=True, stop=True)
            hs = tmp.tile([NP, NL], F32, tag="hs")
            nc.vector.tensor_tensor(hs[:], h[:], hmb[:], OP.subtract)
            ex = tmp.tile([NP, NL], F32, tag="ex")
            nc.scalar.activation(ex[:], hs[:], AF.Exp)
            onesf = cst.tile([NP, 1], F32, tag="onesf")
            nc.vector.memset(onesf[:], 1.0)
            tot = ps1.tile([1, NL], F32, tag="tp1")
            nc.tensor.matmul(tot[:], onesf[:], ex[:], start=True, stop=True)
            ltot = tmp.tile([1, NL], F32, tag="ltot")
            nc.scalar.activation(ltot[:], tot[:], AF.Ln)
            # log of scales, then sum over renorm events (n-major layout)
            lr = tmp.tile([1, n_ren * NL], F32, tag="lr")
            nc.scalar.activation(lr[:], rlog[:], AF.Ln)
            slog = tmp.tile([1, NL], F32, tag="slog")
            nc.vector.tensor_reduce(
                slog[:],
                _ap(lr, 0, [[n_ren * NL, 1], [n_ren, NL], [1, n_ren]]),
                mybir.AxisListType.X,
                OP.add,
            )
            q = tmp.tile([1, NL], F32, tag="q")
            nc.vector.tensor_add(q[:], ltot[:], slog[:])
            q1 = tmp.tile([1, NL], F32, tag="q1")
            nc.vector.tensor_add(q1[:], q[:], hm[:])
            # (q1 - T*SHIFT) * -1 = T*SHIFT - q1
            q2 = tmp.tile([1, NL], F32, tag="q2")
            nc.vector.tensor_scalar(q2[:], q1[:], float(T) * SHIFT, -1.0, OP.subtract, OP.mult)
            rl = tmp.tile([1, NL], F32, tag="rl")
            nc.vector.reciprocal(rl[:], lrow[:])
            loss = tmp.tile([1, NL], F32, tag="loss")
            nc.vector.tensor_mul(loss[:], q2[:], rl[:])
            nc.sync.dma_start(out_ext[:], loss[:])

    nc.compile()
    return nc


_NC_CACHE = {}


def _get_nc(T=T_FULL):
    if T not in _NC_CACHE:
        _NC_CACHE[T] = build_nc(T)
    return _NC_CACHE[T]


def kernel(log_probs, targets, input_lengths, target_lengths):
    lp = np.ascontiguousarray(np.asarray(log_probs, dtype=np.float32))
    tg = np.ascontiguousarray(np.asarray(targets, dtype=np.int32))
    tl = np.ascontiguousarray(np.asarray(target_lengths, dtype=np.int32))
    T, N, _ = lp.shape
    nc = _get_nc(T)
    in_maps = []
    for i in range(NC_CORES):
        s = slice(i * NL, (i + 1) * NL)
        in_maps.append(
            {
                "log_probs": np.ascontiguousarray(lp[:, s, :]),
                "targets": np.ascontiguousarray(tg[s]),
                "target_lengths": np.ascontiguousarray(tl[s]),
            }
        )
    res = run_bass_kernel_spmd(nc, in_maps, core_ids=list(range(NC_CORES)))
    out = np.concatenate([res.results[i]["out"].reshape(NL) for i in range(NC_CORES)])
    return out.astype(np.float32)
